# revision 5
# baseline (speedup 1.0000x reference)
"""Trainium2 Bass kernel for nn_MultiHeadAttention_68659347194437.

Spatial multi-head attention over the W axis (no softmax):
    qp = wq*q + bq ; kp, vp likewise            (1x1 conv over C=32)
    attn = qp @ kp^T  per (b,h)                 [512, 512]
    att  = attn @ vp                            [512, 32]
    out  = att^T + q                            (NCHW residual)

No softmax => associativity collapses the [512,512] score matrix:
    Graw = V_aug K_aug^T per head               [33,33] (aug = ones row)
    M1   = Graw^T Pv_aug                        [33,32]
    L    = A M1 + [I;0]                         [33,32], A = [wq|bq]^T[wk|bk]
    out  = L^T Q_aug  per head                  (residual + biases folded)

Layout/pipeline strategy (v2):
  - everything lives in SBUF at once (~70KB/partition of 208KB): few giant
    DMAs on the two HWDGE queues (sync=kt+stores, scalar=vt+q), no buffer
    recycling.
  - compute in 8 rounds of 8 heads; gt/m1/l PSUM tiles batch 2 blocks to
    halve DVE copy count.
  - emission is software-pipelined across rounds (GT leads by 3 stages) so
    the PE never waits on the DVE PSUM->SBUF copy chain.

Sharding: data-parallel over batch B=8 across 8 NeuronCores, no comms.
Host-side work is pure relayout (transpose/concat/cast) of inputs.
"""

import os
import numpy as np

import concourse.bass as bass
import concourse.bacc as bacc
import concourse.tile as tile
import concourse.mybir as mybir
from concourse.bass_utils import run_bass_kernel_spmd

B, C, H, W = 8, 32, 64, 512
CA = C + 1          # augmented channel dim (ones row/col)
HW = H * W          # 32768 pixels per (b)
NCHUNK = HW // 128  # 256 chunks of 128 pixels (4 per head)
NB = H // 4         # 16 blocks of 4 heads
NR = H // 8         # 8 rounds of 2 blocks (8 heads)
RC = 8 * 4 * CA     # kt/vt columns per round (1056)

KV_DT = os.environ.get("KERNEL_KV_DT", "bf16")
Q_DT = os.environ.get("KERNEL_Q_DT", "bf16")

_DT = {"f32": mybir.dt.float32, "bf16": mybir.dt.bfloat16}

last_exec_time_ns = None

_cache = {}


def _np_dt(name):
    return np.dtype(np.float32) if name == "f32" else np.dtype(mybir.dt.np(mybir.dt.bfloat16))


# packed f32 constant block: [33, 420]
#   [:, 0:33]    i33 identity (for A transpose)
#   [0:32, 33:66]   wq_aug = [wq|bq] as [32, 33]
#   [0:32, 66:99]   wk_aug = [wk|bk]
#   [:, 99:131]  pvt = [wv^T; bv]  [33, 32]
#   [0:32, 131:164] iext = [I | 0] [32, 33]
#   [0:32, 164:420] i32 = identity tiled 8x [32, 256]
PK_I33 = 0
PK_WQ = 33
PK_WK = 66
PK_PVT = 99
PK_IEXT = 131
PK_I32 = 164
PK_N = 420


def _build(kv_dt_name, q_dt_name):
    kv_dt = _DT[kv_dt_name]
    q_dt = _DT[q_dt_name]
    f32 = mybir.dt.float32

    nc = bacc.Bacc(
        "TRN2",
        target_bir_lowering=False,
        debug=False,
        enable_asserts=False,
        num_devices=8,
    )

    qa_d = nc.dram_tensor("qa", [CA, HW], q_dt, kind="ExternalInput")
    kt_d = nc.dram_tensor("kta", [128, NCHUNK * CA], kv_dt, kind="ExternalInput")
    vt_d = nc.dram_tensor("vta", [128, NCHUNK * CA], kv_dt, kind="ExternalInput")
    pk_d = nc.dram_tensor("packed", [CA, PK_N], f32, kind="ExternalInput")
    out_d = nc.dram_tensor("out", [128, NB * W], kv_dt, kind="ExternalOutput")

    qa = qa_d.ap()
    kta = kt_d.ap()
    vta = vt_d.ap()
    out_ap = out_d.ap()

    with tile.TileContext(nc) as tc:
        with (
            tc.tile_pool(name="sm", bufs=4) as spool,
            tc.tile_pool(name="psg", bufs=2, space=bass.MemorySpace.PSUM) as psg,
            tc.tile_pool(name="psy", bufs=2, space=bass.MemorySpace.PSUM) as psy,
            tc.tile_pool(name="psl", bufs=2, space=bass.MemorySpace.PSUM) as psl,
            tc.tile_pool(name="pso", bufs=2, space=bass.MemorySpace.PSUM) as pso,
        ):
            # ---- persistent SBUF tiles (freed LIFO at the end) ----
            _frees = []

            def _ptile(shape, dtype, name):
                t, free = tc.tile(shape, dtype, name=name)
                _frees.append(free)
                return t

            kt_sb = _ptile([128, NCHUNK * CA], kv_dt, "kt_sb")
            vt_sb = _ptile([128, NCHUNK * CA], kv_dt, "vt_sb")
            qa_sb = _ptile([CA, HW], q_dt, "qa_sb")
            og = _ptile([128, NB * W], kv_dt, "og")
            pk = _ptile([CA, PK_N], f32, "pk")
            pvt = _ptile([CA, C], kv_dt, "pvt")
            iext = _ptile([C, CA], kv_dt, "iext")
            i32 = _ptile([C, 8 * C], kv_dt, "i32")
            a_sb = _ptile([CA, CA], f32, "a_sb")
            at_sb = _ptile([CA, CA], kv_dt, "at_sb")

            # ---- load DMAs: all issued up front, stream continuously ----
            # sync queue: consts, then kt (rounds 0,1,23,45,67)
            nc.sync.dma_start(pk[:], pk_d.ap()[:])
            kt_splits = [(0, 1), (1, 2), (2, 4), (4, 6), (6, 8)]
            for r0, r1 in kt_splits:
                nc.sync.dma_start(kt_sb[:, r0 * RC:r1 * RC], kta[:, r0 * RC:r1 * RC])
            # scalar queue: vt / qa interleaved by need-time
            qa_splits = [(0, 2), (2, 4), (4, 6), (6, 8)]
            qi = 0
            for i, (r0, r1) in enumerate(kt_splits):
                nc.scalar.dma_start(vt_sb[:, r0 * RC:r1 * RC], vta[:, r0 * RC:r1 * RC])
                if i >= 1 and qi < 4:
                    q0, q1 = qa_splits[qi]
                    nc.scalar.dma_start(
                        qa_sb[:, q0 * 8 * W:q1 * 8 * W], qa[:, q0 * 8 * W:q1 * 8 * W]
                    )
                    qi += 1

            # ---- derived constants (off critical path; overlap the loads) ----
            # convert bf16 working copies
            nc.vector.tensor_copy(pvt[:], pk[:, PK_PVT:PK_PVT + C])
            nc.vector.tensor_copy(iext[:], pk[:C, PK_IEXT:PK_IEXT + CA])
            nc.vector.tensor_copy(i32[:], pk[:C, PK_I32:PK_I32 + 8 * C])
            # A = [wq|bq]^T [wk|bk]; AT = A^T via PE transpose with identity
            a_ps = psg.tile([CA, CA], f32, tag="g")
            nc.tensor.matmul(a_ps[:], pk[:C, PK_WQ:PK_WQ + CA], pk[:C, PK_WK:PK_WK + CA])
            nc.vector.tensor_copy(a_sb[:], a_ps[:])
            at_ps = psg.tile([CA, CA], f32, tag="g")
            nc.tensor.matmul(at_ps[:], a_sb[:], pk[:, PK_I33:PK_I33 + CA])
            nc.vector.tensor_copy(at_sb[:], at_ps[:])

            # ---- software-pipelined rounds ----
            # stage lag: GT(e) | gt_sb/M1/m1b(e-1) | seed+L/l_sb(e-2) | OUT/og/store(e-3)
            gt_ps_r = {}
            gt_sb_r = {}
            m1_ps_r = {}
            m1b_r = {}
            l_ps_r = {}
            l_sb_r = {}
            o_ps_b = {}

            for e in range(NR + 3):
                # S0: GT for round e
                if e < NR:
                    gt_ps = psg.tile([CA, 8 * CA], f32, tag="g")
                    for j in range(8):
                        for c in range(4):
                            o = (e * 32 + j * 4 + c) * CA
                            nc.tensor.matmul(
                                gt_ps[:, j * CA:(j + 1) * CA],
                                vt_sb[:, o:o + CA],
                                kt_sb[:, o:o + CA],
                                start=(c == 0),
                                stop=(c == 3),
                            )
                    gt_ps_r[e] = gt_ps

                # S1: gt copy + M1 + m1 copy for round e-1
                r = e - 1
                if 0 <= r < NR:
                    gt_sb = spool.tile([CA, 8 * CA], kv_dt, tag="gt_sb")
                    nc.vector.tensor_copy(gt_sb[:], gt_ps_r.pop(r)[:])
                    m1_ps = psy.tile([CA, 8 * C], f32, tag="m1")
                    for j in range(8):
                        nc.tensor.matmul(
                            m1_ps[:, j * C:(j + 1) * C],
                            gt_sb[:, j * CA:(j + 1) * CA],
                            pvt[:],
                        )
                    m1b = spool.tile([CA, 8 * C], kv_dt, tag="m1b")
                    nc.vector.tensor_copy(m1b[:], m1_ps[:])
                    m1b_r[r] = m1b

                # S2: seed + L + l copy for round e-2
                r = e - 2
                if 0 <= r < NR:
                    l_ps = psl.tile([CA, 8 * C], f32, tag="l")
                    nc.tensor.matmul(l_ps[:], iext[:], i32[:], start=True, stop=False)
                    nc.tensor.matmul(l_ps[:], at_sb[:], m1b_r.pop(r)[:], start=False, stop=True)
                    l_sb = spool.tile([CA, 8 * C], q_dt, tag="l_sb")
                    nc.vector.tensor_copy(l_sb[:], l_ps[:])
                    l_sb_r[r] = l_sb

                # S3: OUT matmuls + og copy + store for round e-3
                r = e - 3
                if 0 <= r < NR:
                    l_sb = l_sb_r.pop(r)
                    for blk_local in range(2):
                        blk = r * 2 + blk_local
                        o_ps = pso.tile([128, W], f32, tag="o")
                        for i in range(4):
                            j = blk_local * 4 + i
                            h = r * 8 + j
                            nc.tensor.matmul(
                                o_ps[32 * i:32 * (i + 1), :],
                                l_sb[:, j * C:(j + 1) * C],
                                qa_sb[:, h * W:(h + 1) * W],
                                tile_position=(0, 32 * i),
                            )
                        if blk % 2 == 0:
                            nc.vector.tensor_copy(og[:, blk * W:(blk + 1) * W], o_ps[:])
                        else:
                            nc.scalar.copy(og[:, blk * W:(blk + 1) * W], o_ps[:])
                    if r % 2 == 1:
                        base = (r - 1) * 2 * W
                        nc.sync.dma_start(
                            out_ap[:, base:base + 4 * W], og[:, base:base + 4 * W]
                        )

            for free in reversed(_frees):
                free()

    nc.compile()
    return nc


def _prep_core(qb, kb, vb, q_np_dt, kv_np_dt):
    """Host-side relayout for one batch element (one core)."""
    qa = np.empty((CA, HW), dtype=q_np_dt)
    qa[:C] = qb.reshape(C, HW)
    qa[C] = 1.0

    def tr(x):
        t = np.empty((HW, CA), dtype=np.float32)
        t[:, :C] = x.reshape(C, HW).T
        t[:, C] = 1.0
        return np.ascontiguousarray(
            t.reshape(NCHUNK, 128, CA).transpose(1, 0, 2)
        ).reshape(128, NCHUNK * CA).astype(kv_np_dt)

    return qa, tr(kb), tr(vb)


def _install_ntff_hook():
    """Provide antenv.axon_hooks (absent in this image) so trace=True works."""
    import sys
    import types

    if "antenv.axon_hooks" in sys.modules:
        return
    try:
        import antenv
    except ImportError:
        return
    mod = types.ModuleType("antenv.axon_hooks")
    store = {}
    mod.set_axon_ntff_profile_hook = lambda h: store.__setitem__("h", h)
    mod.get_axon_ntff_profile_hook = lambda: store.get("h")
    sys.modules["antenv.axon_hooks"] = mod
    antenv.axon_hooks = mod
    try:
        from trn_agent_boot.trn_boot import _ntff_profile_via_ctypes

        hook = _ntff_profile_via_ctypes("/opt/axon/libaxon_pjrt.so")
        if hook is not None:
            store["h"] = hook
    except Exception:
        pass


def kernel(q, k, v, wq, bq, wk, bk, wv, bv):
    global last_exec_time_ns
    key = (KV_DT, Q_DT)
    if key not in _cache:
        _cache[key] = _build(*key)
    nc = _cache[key]

    q_np_dt = _np_dt(Q_DT)
    kv_np_dt = _np_dt(KV_DT)

    q = np.asarray(q, np.float32)
    k = np.asarray(k, np.float32)
    v = np.asarray(v, np.float32)
    wq = np.asarray(wq, np.float32)
    bq = np.asarray(bq, np.float32)
    wk = np.asarray(wk, np.float32)
    bk = np.asarray(bk, np.float32)
    wv = np.asarray(wv, np.float32)
    bv = np.asarray(bv, np.float32)

    packed = np.zeros((CA, PK_N), dtype=np.float32)
    packed[:, PK_I33:PK_I33 + CA] = np.eye(CA)
    packed[:C, PK_WQ:PK_WQ + CA] = np.concatenate([wq, bq[:, None]], axis=1)
    packed[:C, PK_WK:PK_WK + CA] = np.concatenate([wk, bk[:, None]], axis=1)
    packed[:, PK_PVT:PK_PVT + C] = np.concatenate([wv.T, bv[None, :]], axis=0)
    packed[:C, PK_IEXT:PK_IEXT + CA] = np.concatenate(
        [np.eye(C), np.zeros((C, 1))], axis=1
    )
    packed[:C, PK_I32:PK_I32 + 8 * C] = np.tile(np.eye(C), (1, 8))

    in_maps = []
    for b in range(B):
        qa, kta, vta = _prep_core(q[b], k[b], v[b], q_np_dt, kv_np_dt)
        in_maps.append({
            "qa": qa, "kta": kta, "vta": vta, "packed": packed,
        })

    trace = os.environ.get("KERNEL_TRACE", "0") == "1"
    if trace:
        _install_ntff_hook()
    res = run_bass_kernel_spmd(nc, in_maps, core_ids=list(range(B)), trace=trace)
    last_exec_time_ns = res.exec_time_ns

    outs = []
    for b in range(B):
        arr = np.asarray(res.results[b]["out"], dtype=np.float32).reshape(4, C, NB, W)
        outs.append(np.transpose(arr, (1, 2, 0, 3)).reshape(C, H, W))
    return np.stack(outs).astype(np.float32)
